# Initial kernel scaffold
#
"""Trainium2 Bass kernel for nn_Attention_Module (sparse_attention).

Computation per batch b (x_b: [C=256, T=4096] fp32):
    energy = x_b @ x_b^T                      # (256, 256), K=4096
    attn   = softmax(rowmax(energy) - energy) # == exp(mu - e)/Z, mu = rowmin
    out    = gamma * (attn @ x_b) + x_b

Strategy (8 cores, pure data-parallel, 4 batches/core):
  - Host pre-transposes x -> xT so the energy matmul (contraction over T)
    gets t-partition tiles with contiguous DMA; x natural layout feeds the
    second matmul (contraction over C) and the +x residual.
  - Matmuls run as float32r (fp22 multiplies, fp32 accumulate).
  - The +x residual is folded into matmul2 via the modified attention matrix
    A'' = gamma*P^T + diag(Z); out = diag(1/Z) * (A''^T @ x). Using the same
    Z in diag and the post-scale makes the x term exact.
  - Software-pipelined DMA issue: batch b+1 loads are issued before batch b
    stores so the HWDGE ring never blocks loads behind compute-gated stores.
    Whole-tensor transfers (4 MB) keep DMA at peak efficiency.
"""

import numpy as np

B, C, T = 32, 256, 4096
NCORES = 8
NB = B // NCORES  # batches per core
P = 128
KT = T // P  # 32 k-tiles for the energy matmul
TC = T // 512  # 8 t-chunks for matmul2

_CACHE = {}


def _build_nc(variant=None):
    variant = variant or {}
    from contextlib import ExitStack

    import concourse.bacc as bacc
    import concourse.bass as bass
    import concourse.tile as tile
    from concourse import mybir

    f32 = mybir.dt.float32
    f32r = mybir.dt.float32r
    ts = bass.ts

    nc = bacc.Bacc(
        "TRN2",
        target_bir_lowering=False,
        debug=False,
        enable_asserts=False,
        num_devices=NCORES,
    )

    xt_h = nc.dram_tensor("xt", [NB, T, C], f32r, kind="ExternalInput")
    xn_h = nc.dram_tensor("xn", [NB, C, T], mybir.dt.float16, kind="ExternalInput")
    # aux: per-partition row [gamma, pad, pad, pad, identity-row(128)]
    aux_h = nc.dram_tensor("aux", [P, 132], f32, kind="ExternalInput")
    o_h = nc.dram_tensor("o", [NB, C, T], f32, kind="ExternalOutput")

    with tile.TileContext(nc) as tc:
        with ExitStack() as ctx:
            singles = ctx.enter_context(tc.tile_pool(name="singles", bufs=1))
            xt_pool = ctx.enter_context(
                tc.tile_pool(
                    name="xt", bufs=3 if variant.get("xt_bufs3") else 2
                )
            )
            xq_pool = (
                ctx.enter_context(tc.tile_pool(name="xq", bufs=1))
                if variant.get("fast_start")
                else None
            )
            xn_pool = ctx.enter_context(tc.tile_pool(name="xn", bufs=3))
            out_pool = ctx.enter_context(tc.tile_pool(name="out", bufs=2))
            att_pool = ctx.enter_context(tc.tile_pool(name="att", bufs=3))
            small = ctx.enter_context(tc.tile_pool(name="small", bufs=4))
            psum_e = ctx.enter_context(
                tc.tile_pool(name="psum_e", bufs=2, space="PSUM")
            )
            psum_t = ctx.enter_context(
                tc.tile_pool(name="psum_t", bufs=2, space="PSUM")
            )
            psum_o = ctx.enter_context(
                tc.tile_pool(
                    name="psum_o",
                    bufs=6 if variant.get("xbar_pt") else 4,
                    space="PSUM",
                )
            )

            xt_ap = xt_h.ap()
            xn_ap = xn_h.ap()
            o_ap = o_h.ap()

            # aux on the ACT ring so it doesn't delay the first xt load
            aux = singles.tile([P, 132], f32)
            nc.scalar.dma_start(aux[:], aux_h.ap())
            gv = aux[:, 0:1]
            ivg = aux[:, 1:2]
            ident = aux[:, 4:132]

            def issue_loads(b):
                KH = KT // 2
                xt_r = xt_ap[b].rearrange("(k p) c -> p k c", p=P)
                if variant.get("fast_start") and b == 0:
                    # four separate tiles so matmul1 starts after the first
                    # 1 MB lands (Tile tracks deps at tile granularity)
                    KQ = KT // 4
                    qs = []
                    for q in range(4):
                        t_ = xq_pool.tile(
                            [P, KQ, C], f32r, tag=f"xq{q}", name=f"xq{q}"
                        )
                        nc.sync.dma_start(
                            t_[:], xt_r[:, q * KQ : (q + 1) * KQ, :]
                        )
                        qs.append(t_)
                    xt_tiles, kdiv = qs, KQ
                else:
                    xta = xt_pool.tile([P, KH, C], f32r, tag="xta", name="xta")
                    xtb = xt_pool.tile([P, KH, C], f32r, tag="xtb", name="xtb")
                    if variant.get("fine_ends") and b == 0:
                        for t_, off in ((xta, 0), (xtb, KH)):
                            for q in range(2):
                                nc.sync.dma_start(
                                    t_[:, q * (KH // 2) : (q + 1) * (KH // 2), :],
                                    xt_r[
                                        :,
                                        off
                                        + q * (KH // 2) : off
                                        + (q + 1) * (KH // 2),
                                        :,
                                    ],
                                )
                    else:
                        nc.sync.dma_start(xta[:], xt_r[:, :KH, :])
                        nc.sync.dma_start(xtb[:], xt_r[:, KH:, :])
                    xt_tiles, kdiv = [xta, xtb], KH
                xn = xn_pool.tile([P, 2, T], mybir.dt.float16, tag="xn", name="xn")
                nc.sync.dma_start(
                    xn[:], xn_ap[b].rearrange("(m p) t -> p m t", p=P)
                )
                return xt_tiles, kdiv, xn

            tiles = {0: issue_loads(0)}
            pending = None  # (b, At, rZ, xn) awaiting matmul2

            for b in range(NB):
                xt, kdiv, xn = tiles.pop(b)
                if b + 1 < NB:
                    tiles[b + 1] = issue_loads(b + 1)

                # A''^T, laid out [128(j within k-block), k-block, 256(i)]
                At = att_pool.tile(
                    [P, 2, C],
                    mybir.dt.bfloat16
                    if variant.get("xbar_pt")
                    else mybir.dt.float16,
                )
                Zs = small.tile([P, 2], f32)
                Zb = small.tile(
                    [P, 2],
                    mybir.dt.bfloat16
                    if variant.get("xbar_pt")
                    else mybir.dt.float16,
                )
                rZ = small.tile([P, 2], f32)

                for m in range(2):
                    pe = psum_e.tile([P, C], mybir.dt.float32)
                    for k in range(KT):
                        src_t = xt[k // kdiv]
                        kk = k % kdiv
                        nc.tensor.matmul(
                            pe[:],
                            lhsT=src_t[:, kk, ts(m, P)],
                            rhs=src_t[:, kk, :],
                            start=(k == 0),
                            stop=(k == KT - 1),
                        )
                    mu = small.tile([P, 1], f32)
                    nc.vector.tensor_reduce(
                        mu[:], pe[:], axis=mybir.AxisListType.X,
                        op=mybir.AluOpType.min,
                    )
                    if variant.get("xbar_pt"):
                        # P in bf16; transpose via xbar DMA on the ACT ring.
                        # A''T = P^T + diag(W), W = Z/gamma; out scale = 1/W.
                        Pm = small.tile([P, C], mybir.dt.bfloat16, tag="Pm")
                        nc.scalar.activation(
                            Pm[:],
                            pe[:],
                            mybir.ActivationFunctionType.Exp,
                            bias=mu[:],
                            scale=-1.0,
                            accum_out=Zs[:, m : m + 1],
                        )
                        Wv = small.tile([P, 2], f32, tag="Wv", name="Wv")
                        nc.vector.tensor_scalar_mul(
                            Wv[:, m : m + 1], Zs[:, m : m + 1], ivg
                        )
                        nc.vector.tensor_copy(Zb[:, m : m + 1], Wv[:, m : m + 1])
                        nc.vector.reciprocal(rZ[:, m : m + 1], Zb[:, m : m + 1])
                        for k in range(2):
                            nc.scalar.dma_start_transpose(
                                At[:, k, ts(m, P)], Pm[:, ts(k, P)]
                            )
                        dg = small.tile([P, P], mybir.dt.bfloat16, tag="diag")
                        nc.vector.tensor_scalar_mul(dg[:], ident, Wv[:, m : m + 1])
                        nc.vector.tensor_add(
                            At[:, m, ts(m, P)], At[:, m, ts(m, P)], dg[:]
                        )
                    else:
                        Pm = small.tile([P, C], f32, tag="Pm")
                        nc.scalar.activation(
                            Pm[:],
                            pe[:],
                            mybir.ActivationFunctionType.Exp,
                            bias=mu[:],
                            scale=-1.0,
                            accum_out=Zs[:, m : m + 1],
                        )
                        nc.vector.tensor_copy(Zb[:, m : m + 1], Zs[:, m : m + 1])
                        nc.vector.reciprocal(rZ[:, m : m + 1], Zb[:, m : m + 1])
                        for k in range(2):
                            pt = psum_t.tile([P, P], mybir.dt.float32)
                            nc.tensor.transpose(pt[:], Pm[:, ts(k, P)], ident)
                            # A''T[j in k-block, i in m-block] = gamma * P^T
                            nc.scalar.mul(At[:, k, ts(m, P)], pt[:], gv)
                        # diagonal: += diag(Z) (falls in the k == m block)
                        dg = small.tile([P, P], mybir.dt.float16, tag="diag")
                        nc.vector.tensor_scalar_mul(dg[:], ident, Zs[:, m : m + 1])
                        nc.vector.tensor_add(
                            At[:, m, ts(m, P)], At[:, m, ts(m, P)], dg[:]
                        )

                # software-pipeline the PE: run the PREVIOUS batch's matmul2
                # after this batch's matmul1, hiding the A'' build latency.
                this = (b, At, rZ, xn)
                todo = [pending] if pending is not None else []
                if b == NB - 1:
                    todo.append(this)
                    pending = None
                else:
                    pending = this
                for pb, pAt, prZ, pxn in todo:
                    for m in range(2):
                        ot = out_pool.tile([P, T], f32, tag="ot", name="ot")
                        if variant.get("mm2_wruns"):
                            # weight-consecutive runs of 2 so walrus ldw-opt
                            # can elide redundant embedded weight loads
                            for g in range(TC // 2):
                                pos = [
                                    psum_o.tile(
                                        [P, 512], mybir.dt.float32,
                                        name=f"po{j}", tag="po",
                                    )
                                    for j in range(2)
                                ]
                                for k in range(2):
                                    for j in range(2):
                                        nc.tensor.matmul(
                                            pos[j][:],
                                            lhsT=pAt[:, k, ts(m, P)],
                                            rhs=pxn[:, k, ts(2 * g + j, 512)],
                                            start=(k == 0),
                                            stop=(k == 1),
                                        )
                                for j in range(2):
                                    t8 = 2 * g + j
                                    if t8 % 2 == 0:
                                        nc.vector.tensor_scalar_mul(
                                            ot[:, ts(t8, 512)], pos[j][:],
                                            prZ[:, m : m + 1],
                                        )
                                    else:
                                        nc.scalar.mul(
                                            ot[:, ts(t8, 512)], pos[j][:],
                                            prZ[:, m : m + 1],
                                        )
                        else:
                            for t8 in range(TC):
                                po = psum_o.tile([P, 512], mybir.dt.float32)
                                for k in range(2):
                                    nc.tensor.matmul(
                                        po[:],
                                        lhsT=pAt[:, k, ts(m, P)],
                                        rhs=pxn[:, k, ts(t8, 512)],
                                        start=(k == 0),
                                        stop=(k == 1),
                                    )
                                # out = psum * (1/Z); alternate engines
                                if t8 % 2 == 0:
                                    nc.vector.tensor_scalar_mul(
                                        ot[:, ts(t8, 512)], po[:], prZ[:, m : m + 1]
                                    )
                                else:
                                    nc.scalar.mul(
                                        ot[:, ts(t8, 512)], po[:], prZ[:, m : m + 1]
                                    )
                        nsplit = (
                            4
                            if (variant.get("fine_ends") or variant.get("fast_start"))
                            and pb == NB - 1
                            else 2
                        )
                        store_eng = (
                            nc.scalar if variant.get("act_stores") else nc.sync
                        )
                        for sh in range(nsplit):
                            store_eng.dma_start(
                                o_ap[pb].rearrange("(m p) t -> p m t", p=P)[
                                    :, m, ts(sh, T // nsplit)
                                ],
                                ot[:, ts(sh, T // nsplit)],
                            )

    nc.compile()
    return nc


def _get_nc():
    if "nc" not in _CACHE:
        _CACHE["nc"] = _build_nc()
    return _CACHE["nc"]


def _make_aux(gamma_val):
    aux = np.zeros((P, 132), dtype=np.float32)
    aux[:, 0] = gamma_val
    aux[:, 1] = 1.0 / gamma_val if gamma_val != 0 else 0.0
    aux[:, 4:132] = np.eye(P, dtype=np.float32)
    return aux


def kernel(x, gamma, _trace=False):
    import concourse.bass_utils as bass_utils

    x = np.ascontiguousarray(np.asarray(x, dtype=np.float32))
    gamma = np.asarray(gamma, dtype=np.float32).reshape(-1)

    nc = _get_nc()

    aux = _make_aux(gamma[0])
    in_maps = []
    for d in range(NCORES):
        xs = x[d * NB : (d + 1) * NB]
        in_maps.append(
            {
                "xt": np.ascontiguousarray(xs.transpose(0, 2, 1)),
                "xn": xs.astype(np.float16),
                "aux": aux,
            }
        )

    res = bass_utils.run_bass_kernel_spmd(
        nc, in_maps, core_ids=list(range(NCORES)), trace=_trace
    )
    out = np.concatenate([r["o"] for r in res.results], axis=0)
    if _trace:
        _CACHE["last_results"] = res
    return out



# revision 2
# speedup vs baseline: 1.6682x; 1.6682x over previous
"""Trainium2 Bass kernel for nn_Attention_Module (sparse_attention).

Computation per batch b (x_b: [C=256, T=4096] fp32):
    energy = x_b @ x_b^T                      # (256, 256), K=4096
    attn   = softmax(rowmax(energy) - energy) # == exp(mu - e)/Z, mu = rowmin
    out    = gamma * (attn @ x_b) + x_b

Strategy (8 cores, pure data-parallel, 4 batches/core):
  - Host pre-swizzles both x layouts so every DMA transfer is a fat
    contiguous run (16-32 KB per partition): 1 KB-granular descriptors
    previously made HWDGE dispatch the bottleneck (~10 us per 2 MB load).
  - xt (t-on-partition, fp32) feeds the energy matmul as float32r;
    xn (c-on-partition, fp16) feeds the second matmul and the residual.
  - The +x residual is folded into matmul2 via the modified attention matrix
    A'' = gamma*P^T + diag(Z); out = diag(1/Z) * (A''^T @ x).
  - Output is stored fp16 (tolerance is 2e-2); host upcasts to fp32.
"""

import numpy as np

B, C, T = 32, 256, 4096
NCORES = 8
NB = B // NCORES  # batches per core
P = 128
KT = T // P  # 32 k-tiles for the energy matmul
TC = T // 512  # 8 t-chunks for matmul2

_CACHE = {}


def _build_nc(variant=None):
    variant = variant or {}
    from contextlib import ExitStack

    import concourse.bacc as bacc
    import concourse.bass as bass
    import concourse.tile as tile
    from concourse import mybir

    f32 = mybir.dt.float32
    f32r = mybir.dt.float32r
    f16 = mybir.dt.float16
    ts = bass.ts

    nc = bacc.Bacc(
        "TRN2",
        target_bir_lowering=False,
        debug=False,
        enable_asserts=False,
        num_devices=NCORES,
    )

    # host-swizzled layouts: contiguous fat runs per partition
    xt_h = nc.dram_tensor("xt", [NB, P, KT * C], f32r, kind="ExternalInput")
    xn_h = nc.dram_tensor("xn", [NB, P, 2 * T], f16, kind="ExternalInput")
    # aux: per-partition row [gamma, pad, pad, pad, identity-row(128)]
    aux_h = nc.dram_tensor("aux", [P, 132], f32, kind="ExternalInput")
    o_h = nc.dram_tensor("o", [NB, P, 2 * T], f16, kind="ExternalOutput")

    with tile.TileContext(nc) as tc:
        with ExitStack() as ctx:
            singles = ctx.enter_context(tc.tile_pool(name="singles", bufs=1))
            xt_pool = ctx.enter_context(tc.tile_pool(name="xt", bufs=2))
            xq_pool = ctx.enter_context(tc.tile_pool(name="xq", bufs=1))
            xn_pool = ctx.enter_context(tc.tile_pool(name="xn", bufs=3))
            out_pool = ctx.enter_context(tc.tile_pool(name="out", bufs=2))
            att_pool = ctx.enter_context(tc.tile_pool(name="att", bufs=3))
            small = ctx.enter_context(tc.tile_pool(name="small", bufs=4))
            psum_e = ctx.enter_context(
                tc.tile_pool(name="psum_e", bufs=2, space="PSUM")
            )
            psum_t = ctx.enter_context(
                tc.tile_pool(name="psum_t", bufs=2, space="PSUM")
            )
            psum_o = ctx.enter_context(
                tc.tile_pool(name="psum_o", bufs=4, space="PSUM")
            )

            xt_ap = xt_h.ap()
            xn_ap = xn_h.ap()
            o_ap = o_h.ap()

            # aux on the ACT ring so it doesn't delay the first xt load
            aux = singles.tile([P, 132], f32)
            nc.scalar.dma_start(aux[:], aux_h.ap())
            gv = aux[:, 0:1]
            ident = aux[:, 4:132]

            KH = KT // 2

            def issue_loads(b):
                if b == 0:
                    # four separate tiles so matmul1 starts after the first
                    # 1 MB lands (Tile tracks deps at tile granularity)
                    KQ = KT // 4
                    qs = []
                    for q in range(4):
                        t_ = xq_pool.tile(
                            [P, KQ, C], f32r, tag=f"xq{q}", name=f"xq{q}"
                        )
                        nc.sync.dma_start(
                            t_[:],
                            xt_ap[b][:, q * KQ * C : (q + 1) * KQ * C],
                        )
                        qs.append(t_)
                    xt_tiles, kdiv = qs, KQ
                else:
                    xta = xt_pool.tile([P, KH, C], f32r, tag="xta", name="xta")
                    xtb = xt_pool.tile([P, KH, C], f32r, tag="xtb", name="xtb")
                    nc.sync.dma_start(xta[:], xt_ap[b][:, : KH * C])
                    nc.sync.dma_start(xtb[:], xt_ap[b][:, KH * C :])
                    xt_tiles, kdiv = [xta, xtb], KH
                xn = xn_pool.tile([P, 2, T], f16, tag="xn", name="xn")
                nc.sync.dma_start(xn[:], xn_ap[b])
                return xt_tiles, kdiv, xn

            tiles = {0: issue_loads(0)}
            pending = None  # (b, At, rZ, xn) awaiting matmul2

            for b in range(NB):
                xt, kdiv, xn = tiles.pop(b)
                if b + 1 < NB:
                    tiles[b + 1] = issue_loads(b + 1)

                # A''^T, laid out [128(j within k-block), k-block, 256(i)]
                At = att_pool.tile([P, 2, C], f16)
                Zs = small.tile([P, 2], f32)
                Zb = small.tile([P, 2], f16)
                rZ = small.tile([P, 2], f32)

                for m in range(2):
                    pe = psum_e.tile([P, C], mybir.dt.float32)
                    for k in range(KT):
                        src_t = xt[k // kdiv]
                        kk = k % kdiv
                        nc.tensor.matmul(
                            pe[:],
                            lhsT=src_t[:, kk, ts(m, P)],
                            rhs=src_t[:, kk, :],
                            start=(k == 0),
                            stop=(k == KT - 1),
                        )
                    mu = small.tile([P, 1], f32)
                    nc.vector.tensor_reduce(
                        mu[:], pe[:], axis=mybir.AxisListType.X,
                        op=mybir.AluOpType.min,
                    )
                    Pm = small.tile([P, C], f32, tag="Pm")
                    nc.scalar.activation(
                        Pm[:],
                        pe[:],
                        mybir.ActivationFunctionType.Exp,
                        bias=mu[:],
                        scale=-1.0,
                        accum_out=Zs[:, m : m + 1],
                    )
                    nc.vector.tensor_copy(Zb[:, m : m + 1], Zs[:, m : m + 1])
                    nc.vector.reciprocal(rZ[:, m : m + 1], Zb[:, m : m + 1])
                    for k in range(2):
                        pt = psum_t.tile([P, P], mybir.dt.float32)
                        nc.tensor.transpose(pt[:], Pm[:, ts(k, P)], ident)
                        # A''T[j in k-block, i in m-block] = gamma * P^T
                        nc.scalar.mul(At[:, k, ts(m, P)], pt[:], gv)
                    # diagonal: += diag(Z) (falls in the k == m block)
                    dg = small.tile([P, P], f16, tag="diag")
                    nc.vector.tensor_scalar_mul(dg[:], ident, Zs[:, m : m + 1])
                    nc.vector.tensor_add(
                        At[:, m, ts(m, P)], At[:, m, ts(m, P)], dg[:]
                    )

                # software-pipeline the PE: run the PREVIOUS batch's matmul2
                # after this batch's matmul1, hiding the A'' build latency.
                this = (b, At, rZ, xn)
                todo = [pending] if pending is not None else []
                if b == NB - 1:
                    todo.append(this)
                    pending = None
                else:
                    pending = this
                for pb, pAt, prZ, pxn in todo:
                    for m in range(2):
                        ot = out_pool.tile([P, T], f16, tag="ot", name="ot")
                        for t8 in range(TC):
                            po = psum_o.tile([P, 512], mybir.dt.float32)
                            for k in range(2):
                                nc.tensor.matmul(
                                    po[:],
                                    lhsT=pAt[:, k, ts(m, P)],
                                    rhs=pxn[:, k, ts(t8, 512)],
                                    start=(k == 0),
                                    stop=(k == 1),
                                )
                            # out = psum * (1/Z); alternate engines
                            if t8 % 2 == 0:
                                nc.vector.tensor_scalar_mul(
                                    ot[:, ts(t8, 512)], po[:], prZ[:, m : m + 1]
                                )
                            else:
                                nc.scalar.mul(
                                    ot[:, ts(t8, 512)], po[:], prZ[:, m : m + 1]
                                )
                        nsplit = 4 if pb == NB - 1 else 2
                        for sh in range(nsplit):
                            nc.sync.dma_start(
                                o_ap[pb][:, m * T :][:, ts(sh, T // nsplit)],
                                ot[:, ts(sh, T // nsplit)],
                            )

    nc.compile()
    return nc


def _get_nc():
    if "nc" not in _CACHE:
        _CACHE["nc"] = _build_nc()
    return _CACHE["nc"]


def _make_aux(gamma_val):
    aux = np.zeros((P, 132), dtype=np.float32)
    aux[:, 0] = gamma_val
    aux[:, 4:132] = np.eye(P, dtype=np.float32)
    return aux


def kernel(x, gamma, _trace=False):
    import concourse.bass_utils as bass_utils

    x = np.ascontiguousarray(np.asarray(x, dtype=np.float32))
    gamma = np.asarray(gamma, dtype=np.float32).reshape(-1)

    nc = _get_nc()

    aux = _make_aux(gamma[0])
    in_maps = []
    for d in range(NCORES):
        xs = x[d * NB : (d + 1) * NB]
        # xt[b, p, k*C+c] = x[b, c, k*128+p]  (fat contiguous runs)
        xt = np.ascontiguousarray(
            xs.transpose(0, 2, 1)
            .reshape(NB, KT, P, C)
            .transpose(0, 2, 1, 3)
            .reshape(NB, P, KT * C)
        )
        # xn[b, p, m*T+t] = x[b, m*128+p, t]
        xn = np.ascontiguousarray(
            xs.reshape(NB, 2, P, T).transpose(0, 2, 1, 3).reshape(NB, P, 2 * T)
        ).astype(np.float16)
        in_maps.append({"xt": xt, "xn": xn, "aux": aux})

    res = bass_utils.run_bass_kernel_spmd(
        nc, in_maps, core_ids=list(range(NCORES)), trace=_trace
    )
    # o[b, p, m*T+t] = out[b, m*128+p, t]
    out = np.concatenate(
        [
            r["o"].reshape(NB, P, 2, T).transpose(0, 2, 1, 3).reshape(NB, C, T)
            for r in res.results
        ],
        axis=0,
    ).astype(np.float32)
    if _trace:
        _CACHE["last_results"] = res
    return out
